# revision 8
# baseline (speedup 1.0000x reference)
"""ClusterGCN on 8 trn2 cores — dense-S formulation, tight per-slot caps.

v2 over v1: per-cluster-slot capacity (ndcap_j exact, nscap_j 128-mult)
instead of one rectangular cap, src-only nodes emitted through the plain
stream (no separate outs tensor), flat-packed xts/st.  Cuts per-device
DMA from ~22.7MB to ~15.4MB.
"""

import numpy as np

N = 100000
D = 256
C = 64
M = 8  # cores

_CHUNK = 2048  # plain-region rows per load/store DMA (1MB bf16)


def _build_program(ndcap, nscap, MAIN_ROWS, chunks, xoff, soff, xsoff):
    import concourse.bacc as bacc
    import concourse.mybir as mybir
    from concourse import tile

    f32 = mybir.dt.float32
    bf16 = mybir.dt.bfloat16
    add = mybir.AluOpType.add
    DSTR = int(sum(ndcap))
    XTS_COLS = int(xoff[8])
    ST_COLS = int(soff[8])
    XS_COLS = int(xsoff[8])
    nd_max = int(max(max(ndcap), 1))
    nsk_max = int(max(max(c // 128 for c in nscap), 1))

    nc = bacc.Bacc("TRN2", target_bir_lowering=False, debug=False, num_devices=M)

    XTM = nc.dram_tensor("xtm", [128, 2 * MAIN_ROWS], bf16, kind="ExternalInput")
    XTS = nc.dram_tensor("xts", [128, XTS_COLS], bf16, kind="ExternalInput")
    ST = nc.dram_tensor("st", [128, ST_COLS], bf16, kind="ExternalInput")
    W = nc.dram_tensor("w", [128, 2, D], bf16, kind="ExternalInput")
    BF = nc.dram_tensor("biasf", [128, 2], f32, kind="ExternalInput")
    OUTT = nc.dram_tensor("outt", [128, 2 * MAIN_ROWS], bf16, kind="ExternalOutput")

    with tile.TileContext(nc) as tc:
        with (
            tc.tile_pool(name="const", bufs=1) as cpool,
            tc.tile_pool(name="xtm", bufs=len(chunks)) as xpool,
            tc.tile_pool(name="outt", bufs=3) as opool,
            tc.tile_pool(name="ps", bufs=3, space="PSUM") as ppool,
        ):
            # scalar HWDGE ring: small consts + S^T; sync ring: the big
            # node streams.  Both rings drain concurrently.
            w_sb = cpool.tile([128, 2, D], bf16)
            nc.scalar.dma_start(w_sb[:], W[:])
            bf_sb = cpool.tile([128, 2], f32)
            nc.scalar.dma_start(bf_sb[:], BF[:])
            xts_sb = cpool.tile([128, XTS_COLS], bf16)
            nc.sync.dma_start(xts_sb[:], XTS[:])
            st_sb = cpool.tile([128, ST_COLS], bf16)
            nc.scalar.dma_start(st_sb[:], ST[:])

            xt_tiles = []
            for idx, (r0, L) in enumerate(chunks):
                xt = xpool.tile([128, 2 * L], bf16, tag="xt")
                nc.sync.dma_start(xt[:], XTM[:, 2 * r0 : 2 * r0 + 2 * L])
                xt_tiles.append(xt)

            xsrc_sb = cpool.tile([128, XS_COLS], bf16)

            # PE warmup: ~3.4us of dummy matmuls on the (tiny, already
            # loaded) weight tile so the HAM clock-gate releases to
            # 2.4GHz before the real stream begins.  One accumulation
            # group into a scratch bank that is never read.
            pw = ppool.tile([128, 512], f32, tag="ps")
            for i in range(16):
                nc.tensor.matmul(
                    pw[:, :D],
                    w_sb[:, 0, 0:128],
                    w_sb[:, i % 2, :],
                    start=(i == 0),
                    stop=(i == 15),
                )

            # ---- src pass: Xl_src = X_src @ W, X-stationary ----
            for j in range(8):
                nsk = nscap[j] // 128
                if nsk == 0:
                    continue
                ps = ppool.tile([128, nsk_max, D], f32, tag="ps")
                for sk in range(nsk):
                    for k in range(2):
                        nc.tensor.matmul(
                            ps[:, sk, :],
                            xts_sb[
                                :,
                                xoff[j] + k * nscap[j] + sk * 128 :
                                xoff[j] + k * nscap[j] + (sk + 1) * 128,
                            ],
                            w_sb[:, k, :],
                            start=(k == 0),
                            stop=(k == 1),
                        )
                xv = xsrc_sb[:, xsoff[j] : xsoff[j] + nsk * D]
                pv = ps[:, :nsk, :]
                if j % 3 == 2:
                    nc.scalar.copy(xv, pv)
                else:
                    nc.vector.tensor_copy(xv, pv)

            eng = 0

            def epi(dst_ap, ps_ap, ft):
                nonlocal eng
                eng += 1
                if eng % 3 == 0:
                    nc.scalar.add(dst_ap, ps_ap, bf_sb[:, ft : ft + 1])
                else:
                    nc.vector.tensor_scalar(
                        dst_ap, ps_ap, bf_sb[:, ft : ft + 1], None, add
                    )

            # ---- main stream ----
            for ci, (r0, L) in enumerate(chunks):
                xt = xt_tiles[ci]
                ot = opool.tile([128, 2 * L], bf16, tag="ot")
                pos = r0
                while pos < r0 + L:
                    off = pos - r0
                    if pos < DSTR:
                        j = next(
                            jj for jj in range(8)
                            if sum(ndcap[:jj]) == pos
                        )
                        nd = ndcap[j]
                        nsk = nscap[j] // 128
                        # each ft slice must be bank-aligned: a matmul
                        # output AP may not cross a 2KB PSUM bank boundary
                        psd = ppool.tile([128, 2, 512], f32, tag="ps")
                        for ft in range(2):
                            for k in range(2):
                                nc.tensor.matmul(
                                    psd[:, ft, :nd],
                                    w_sb[:, k, ft * 128 : (ft + 1) * 128],
                                    xt[:, k * L + off : k * L + off + nd],
                                    start=(k == 0),
                                    stop=(nsk == 0 and k == 1),
                                )
                            for sk in range(nsk):
                                nc.tensor.matmul(
                                    psd[:, ft, :nd],
                                    xsrc_sb[
                                        :,
                                        xsoff[j] + sk * D + ft * 128 :
                                        xsoff[j] + sk * D + (ft + 1) * 128,
                                    ],
                                    st_sb[
                                        :,
                                        soff[j] + sk * nd : soff[j] + (sk + 1) * nd,
                                    ],
                                    start=False,
                                    stop=(sk == nsk - 1),
                                )
                            epi(
                                ot[:, ft * L + off : ft * L + off + nd],
                                psd[:, ft, :nd],
                                ft,
                            )
                        pos += nd
                    else:
                        n = min(512, r0 + L - pos)
                        ps = ppool.tile([128, 2, 512], f32, tag="ps")
                        for ft in range(2):
                            for k in range(2):
                                nc.tensor.matmul(
                                    ps[:, ft, :n],
                                    w_sb[:, k, ft * 128 : (ft + 1) * 128],
                                    xt[:, k * L + off : k * L + off + n],
                                    start=(k == 0),
                                    stop=(k == 1),
                                )
                            epi(
                                ot[:, ft * L + off : ft * L + off + n],
                                ps[:, ft, :n],
                                ft,
                            )
                        pos += n
                nc.scalar.dma_start(OUTT[:, 2 * r0 : 2 * r0 + 2 * L], ot[:])

    nc.compile()
    return nc


def _run_program(nc, in_maps):
    from concourse.bass_utils import run_bass_kernel_spmd

    return run_bass_kernel_spmd(nc, in_maps, core_ids=list(range(M))).results


def _ceil_to(x, m):
    return -(-x // m) * m


def _pack_rows(rows_bf16, chunks):
    out = np.empty((128, 2 * rows_bf16.shape[0]), dtype=rows_bf16.dtype)
    for r0, L in chunks:
        seg = rows_bf16[r0 : r0 + L]
        out[:, 2 * r0 : 2 * r0 + 2 * L] = (
            seg.reshape(L, 2, 128).transpose(2, 1, 0).reshape(128, 2 * L)
        )
    return out


def _unpack_rows(packed, chunks, main_rows):
    rows = np.empty((main_rows, 256), dtype=np.float32)
    for r0, L in chunks:
        blk = packed[:, 2 * r0 : 2 * r0 + 2 * L].reshape(128, 2, L)
        rows[r0 : r0 + L] = (
            blk.transpose(2, 1, 0).reshape(L, 256).astype(np.float32)
        )
    return rows


def kernel(X, weight, bias, cluster_assignment, edge_index):
    import ml_dtypes

    bf = ml_dtypes.bfloat16
    X = np.ascontiguousarray(np.asarray(X, dtype=np.float32))
    weight = np.ascontiguousarray(np.asarray(weight, dtype=np.float32))
    bias = np.asarray(bias, dtype=np.float32)
    cl = np.asarray(cluster_assignment).astype(np.int64)
    ei = np.asarray(edge_index).astype(np.int64)

    src, dst = ei[0], ei[1]
    intra = cl[src] == cl[dst]
    es, ed = src[intra], dst[intra]

    deg = (np.bincount(ed, minlength=N) + 1.0).astype(np.float32)
    dinv = (1.0 / np.sqrt(deg)).astype(np.float32)

    # clusters -> devices: snake over size-sorted clusters, 8 per device
    csize = np.bincount(cl, minlength=C)
    order = np.argsort(-csize, kind="stable")
    cdev = np.zeros(C, dtype=np.int64)
    for i, c in enumerate(order):
        r, q = divmod(i, M)
        cdev[c] = q if r % 2 == 0 else M - 1 - q

    # group intra edges by cluster
    ecl = cl[ed]
    eorder = np.argsort(ecl, kind="stable")
    es_s, ed_s = es[eorder], ed[eorder]
    cstarts = np.searchsorted(ecl[eorder], np.arange(C + 1))

    clusters = {}  # c -> (dst_u, src_u, S [nd, ns])
    for c in range(C):
        a, b = cstarts[c], cstarts[c + 1]
        eds, ess = ed_s[a:b], es_s[a:b]
        dst_u, di = np.unique(eds, return_inverse=True)
        src_u, si = np.unique(ess, return_inverse=True)
        S = np.zeros((dst_u.size, src_u.size), dtype=np.float32)
        # compensate prescale of dst-block rows?  No: sources come from
        # xts (unscaled copies), plain norm applies.
        np.add.at(S, (di, si), dinv[eds] * dinv[ess])
        clusters[c] = (dst_u, src_u, S)

    # within each device sort clusters by workload desc -> slots
    dev_clusters = [[] for _ in range(M)]
    for c in range(C):
        dev_clusters[cdev[c]].append(c)
    for d in range(M):
        dev_clusters[d].sort(
            key=lambda c: -(clusters[c][0].size + clusters[c][1].size)
        )

    ndcap = [0] * 8
    nscap = [0] * 8
    for j in range(8):
        ndcap[j] = max(clusters[dev_clusters[d][j]][0].size for d in range(M))
        nsmax = max(clusters[dev_clusters[d][j]][1].size for d in range(M))
        nscap[j] = _ceil_to(nsmax, 128) if nsmax else 0
    assert max(ndcap) <= 512 and max(nscap) <= 512, (ndcap, nscap)

    DSTR = int(sum(ndcap))
    xoff = np.concatenate([[0], np.cumsum([2 * s for s in nscap])]).astype(int)
    soff = np.concatenate(
        [[0], np.cumsum([(nscap[j] // 128) * ndcap[j] for j in range(8)])]
    ).astype(int)
    xsoff = np.concatenate(
        [[0], np.cumsum([(nscap[j] // 128) * D for j in range(8)])]
    ).astype(int)

    # plain nodes: all device nodes that are not a dst of their cluster
    is_dst = np.zeros(N, dtype=bool)
    for c in range(C):
        is_dst[clusters[c][0]] = True
    node_dev = cdev[cl]
    plain_lists = [
        np.where((node_dev == d) & ~is_dst)[0] for d in range(M)
    ]
    max_plain = max(p.size for p in plain_lists)
    MAIN_ROWS = DSTR + _ceil_to(max(max_plain, 1), 512)

    # split the dst region at a cluster boundary so the first dst
    # matmuls only wait for half the region's load
    half = int(sum(ndcap[:4]))
    chunks = [(0, half), (half, DSTR - half)] if 0 < half < DSTR else [(0, DSTR)]
    r0 = DSTR
    while r0 < MAIN_ROWS:
        L = min(_CHUNK, MAIN_ROWS - r0)
        chunks.append((r0, L))
        r0 += L

    w_pack = np.ascontiguousarray(
        weight.reshape(2, 128, D).transpose(1, 0, 2).astype(bf)
    )
    biasf = np.ascontiguousarray(bias.reshape(2, 128).T.astype(np.float32))

    in_maps = []
    main_ids = []
    for d in range(M):
        rows = np.zeros((MAIN_ROWS, D), dtype=np.float32)
        mid = np.full(MAIN_ROWS, -1, dtype=np.int64)
        xts = np.zeros((128, int(xoff[8])), dtype=bf)
        st = np.zeros((128, int(soff[8])), dtype=bf)
        pos = 0
        for j in range(8):
            c = dev_clusters[d][j]
            dst_u, src_u, S = clusters[c]
            nd, ns = dst_u.size, src_u.size
            rows[pos : pos + nd] = X[dst_u] * (dinv[dst_u] ** 2)[:, None]
            mid[pos : pos + nd] = dst_u
            pos += ndcap[j]
            if nscap[j]:
                xs = np.zeros((nscap[j], D), dtype=np.float32)
                xs[:ns] = X[src_u]
                xts[:, xoff[j] : xoff[j + 1]] = (
                    xs.reshape(nscap[j], 2, 128)
                    .transpose(2, 1, 0)
                    .reshape(128, 2 * nscap[j])
                )
                Sp = np.zeros((nscap[j], ndcap[j]), dtype=np.float32)
                Sp[:ns, :nd] = S.T
                st[:, soff[j] : soff[j + 1]] = (
                    Sp.reshape(nscap[j] // 128, 128, ndcap[j])
                    .transpose(1, 0, 2)
                    .reshape(128, (nscap[j] // 128) * ndcap[j])
                )
        plain = plain_lists[d]
        rows[DSTR : DSTR + plain.size] = X[plain]
        mid[DSTR : DSTR + plain.size] = plain
        main_ids.append(mid)
        in_maps.append({
            "xtm": _pack_rows(np.ascontiguousarray(rows.astype(bf)), chunks),
            "xts": np.ascontiguousarray(xts.astype(bf)),
            "st": np.ascontiguousarray(st.astype(bf)),
            "w": w_pack,
            "biasf": biasf,
        })

    nc = _build_program(
        ndcap, nscap, MAIN_ROWS, chunks, xoff, soff, xsoff
    )
    results = _run_program(nc, in_maps)

    epc = np.bincount(cl[ed], minlength=C)
    active = epc[cl] > 0

    out = X.copy()
    for d in range(M):
        rows = _unpack_rows(np.asarray(results[d]["outt"]), chunks, MAIN_ROWS)
        mid = main_ids[d]
        sel = mid >= 0
        ids = mid[sel]
        act = active[ids]
        out[ids[act]] = rows[sel][act]
    return out


# revision 10
# speedup vs baseline: 1.0317x; 1.0317x over previous
"""ClusterGCN on 8 trn2 cores — dense-S formulation, tight per-slot caps.

v2 over v1: per-cluster-slot capacity (ndcap_j exact, nscap_j 128-mult)
instead of one rectangular cap, src-only nodes emitted through the plain
stream (no separate outs tensor), flat-packed xts/st.  Cuts per-device
DMA from ~22.7MB to ~15.4MB.
"""

import numpy as np

N = 100000
D = 256
C = 64
M = 8  # cores

_CHUNK = 2048  # plain-region rows per load/store DMA (1MB bf16)


def _build_program(ndcap, nscap, MAIN_ROWS, chunks, xoff, soff, xsoff):
    import concourse.bacc as bacc
    import concourse.mybir as mybir
    from concourse import tile

    f32 = mybir.dt.float32
    bf16 = mybir.dt.bfloat16
    add = mybir.AluOpType.add
    DSTR = int(sum(ndcap))
    XTS_COLS = int(xoff[8])
    ST_COLS = int(soff[8])
    XS_COLS = int(xsoff[8])
    nd_max = int(max(max(ndcap), 1))
    nsk_max = int(max(max(c // 128 for c in nscap), 1))

    nc = bacc.Bacc("TRN2", target_bir_lowering=False, debug=False, num_devices=M)

    XTM = nc.dram_tensor("xtm", [128, 2 * MAIN_ROWS], bf16, kind="ExternalInput")
    XTS = nc.dram_tensor("xts", [128, XTS_COLS], bf16, kind="ExternalInput")
    ST = nc.dram_tensor("st", [128, ST_COLS], bf16, kind="ExternalInput")
    W = nc.dram_tensor("w", [128, 2, D], bf16, kind="ExternalInput")
    BF = nc.dram_tensor("biasf", [128, 2], f32, kind="ExternalInput")
    OUTT = nc.dram_tensor("outt", [128, 2 * MAIN_ROWS], bf16, kind="ExternalOutput")

    with tile.TileContext(nc) as tc:
        with (
            tc.tile_pool(name="const", bufs=1) as cpool,
            tc.tile_pool(name="xtm", bufs=len(chunks)) as xpool,
            tc.tile_pool(name="outt", bufs=4) as opool,
            tc.tile_pool(name="ps", bufs=3, space="PSUM") as ppool,
        ):
            # all loads on the sync HWDGE ring in dependency order (the
            # scalar ring starts ~3us late behind the ACT-table preamble);
            # stores go on the scalar ring.
            w_sb = cpool.tile([128, 2, D], bf16)
            nc.sync.dma_start(w_sb[:], W[:])
            bf_sb = cpool.tile([128, 2], f32)
            nc.sync.dma_start(bf_sb[:], BF[:])
            xts_sb = cpool.tile([128, XTS_COLS], bf16)
            nc.sync.dma_start(xts_sb[:], XTS[:])
            st_sb = cpool.tile([128, ST_COLS], bf16)

            xt_tiles = []
            for idx, (r0, L) in enumerate(chunks):
                xt = xpool.tile([128, 2 * L], bf16, tag="xt")
                nc.sync.dma_start(xt[:], XTM[:, 2 * r0 : 2 * r0 + 2 * L])
                xt_tiles.append(xt)
                if idx == 0:
                    # S^T right after the first dst half-chunk
                    nc.sync.dma_start(st_sb[:], ST[:])

            xsrc_sb = cpool.tile([128, XS_COLS], bf16)

            # PE warmup: ~3.4us of dummy matmuls on the (tiny, already
            # loaded) weight tile so the HAM clock-gate releases to
            # 2.4GHz before the real stream begins.  One accumulation
            # group into a scratch bank that is never read.
            pw = ppool.tile([128, 512], f32, tag="ps")
            for i in range(16):
                nc.tensor.matmul(
                    pw[:, :D],
                    w_sb[:, 0, 0:128],
                    w_sb[:, i % 2, :],
                    start=(i == 0),
                    stop=(i == 15),
                )

            # ---- src pass: Xl_src = X_src @ W, X-stationary ----
            for j in range(8):
                nsk = nscap[j] // 128
                if nsk == 0:
                    continue
                ps = ppool.tile([128, nsk_max, D], f32, tag="ps")
                for sk in range(nsk):
                    for k in range(2):
                        nc.tensor.matmul(
                            ps[:, sk, :],
                            xts_sb[
                                :,
                                xoff[j] + k * nscap[j] + sk * 128 :
                                xoff[j] + k * nscap[j] + (sk + 1) * 128,
                            ],
                            w_sb[:, k, :],
                            start=(k == 0),
                            stop=(k == 1),
                        )
                xv = xsrc_sb[:, xsoff[j] : xsoff[j] + nsk * D]
                pv = ps[:, :nsk, :]
                if j % 3 == 2:
                    nc.scalar.copy(xv, pv)
                else:
                    nc.vector.tensor_copy(xv, pv)

            eng = 0

            def epi(dst_ap, ps_ap, ft):
                nonlocal eng
                eng += 1
                if eng % 3 == 0:
                    nc.scalar.add(dst_ap, ps_ap, bf_sb[:, ft : ft + 1])
                else:
                    nc.vector.tensor_scalar(
                        dst_ap, ps_ap, bf_sb[:, ft : ft + 1], None, add
                    )

            # ---- main stream ----
            for ci, (r0, L) in enumerate(chunks):
                xt = xt_tiles[ci]
                ot = opool.tile([128, 2 * L], bf16, tag="ot")
                pos = r0
                while pos < r0 + L:
                    off = pos - r0
                    if pos < DSTR:
                        j = next(
                            jj for jj in range(8)
                            if sum(ndcap[:jj]) == pos
                        )
                        nd = ndcap[j]
                        nsk = nscap[j] // 128
                        # each ft slice must be bank-aligned: a matmul
                        # output AP may not cross a 2KB PSUM bank boundary
                        psd = ppool.tile([128, 2, 512], f32, tag="ps")
                        for ft in range(2):
                            for k in range(2):
                                nc.tensor.matmul(
                                    psd[:, ft, :nd],
                                    w_sb[:, k, ft * 128 : (ft + 1) * 128],
                                    xt[:, k * L + off : k * L + off + nd],
                                    start=(k == 0),
                                    stop=(nsk == 0 and k == 1),
                                )
                            for sk in range(nsk):
                                nc.tensor.matmul(
                                    psd[:, ft, :nd],
                                    xsrc_sb[
                                        :,
                                        xsoff[j] + sk * D + ft * 128 :
                                        xsoff[j] + sk * D + (ft + 1) * 128,
                                    ],
                                    st_sb[
                                        :,
                                        soff[j] + sk * nd : soff[j] + (sk + 1) * nd,
                                    ],
                                    start=False,
                                    stop=(sk == nsk - 1),
                                )
                            epi(
                                ot[:, ft * L + off : ft * L + off + nd],
                                psd[:, ft, :nd],
                                ft,
                            )
                        pos += nd
                    else:
                        n = min(512, r0 + L - pos)
                        ps = ppool.tile([128, 2, 512], f32, tag="ps")
                        for ft in range(2):
                            for k in range(2):
                                nc.tensor.matmul(
                                    ps[:, ft, :n],
                                    w_sb[:, k, ft * 128 : (ft + 1) * 128],
                                    xt[:, k * L + off : k * L + off + n],
                                    start=(k == 0),
                                    stop=(k == 1),
                                )
                            epi(
                                ot[:, ft * L + off : ft * L + off + n],
                                ps[:, ft, :n],
                                ft,
                            )
                        pos += n
                nc.scalar.dma_start(OUTT[:, 2 * r0 : 2 * r0 + 2 * L], ot[:])

    nc.compile()
    return nc


def _run_program(nc, in_maps):
    from concourse.bass_utils import run_bass_kernel_spmd

    return run_bass_kernel_spmd(nc, in_maps, core_ids=list(range(M))).results


def _ceil_to(x, m):
    return -(-x // m) * m


def _pack_rows(rows_bf16, chunks):
    out = np.empty((128, 2 * rows_bf16.shape[0]), dtype=rows_bf16.dtype)
    for r0, L in chunks:
        seg = rows_bf16[r0 : r0 + L]
        out[:, 2 * r0 : 2 * r0 + 2 * L] = (
            seg.reshape(L, 2, 128).transpose(2, 1, 0).reshape(128, 2 * L)
        )
    return out


def _unpack_rows(packed, chunks, main_rows):
    rows = np.empty((main_rows, 256), dtype=np.float32)
    for r0, L in chunks:
        blk = packed[:, 2 * r0 : 2 * r0 + 2 * L].reshape(128, 2, L)
        rows[r0 : r0 + L] = (
            blk.transpose(2, 1, 0).reshape(L, 256).astype(np.float32)
        )
    return rows


def kernel(X, weight, bias, cluster_assignment, edge_index):
    import ml_dtypes

    bf = ml_dtypes.bfloat16
    X = np.ascontiguousarray(np.asarray(X, dtype=np.float32))
    weight = np.ascontiguousarray(np.asarray(weight, dtype=np.float32))
    bias = np.asarray(bias, dtype=np.float32)
    cl = np.asarray(cluster_assignment).astype(np.int64)
    ei = np.asarray(edge_index).astype(np.int64)

    src, dst = ei[0], ei[1]
    intra = cl[src] == cl[dst]
    es, ed = src[intra], dst[intra]

    deg = (np.bincount(ed, minlength=N) + 1.0).astype(np.float32)
    dinv = (1.0 / np.sqrt(deg)).astype(np.float32)

    # clusters -> devices: snake over size-sorted clusters, 8 per device
    csize = np.bincount(cl, minlength=C)
    order = np.argsort(-csize, kind="stable")
    cdev = np.zeros(C, dtype=np.int64)
    for i, c in enumerate(order):
        r, q = divmod(i, M)
        cdev[c] = q if r % 2 == 0 else M - 1 - q

    # group intra edges by cluster
    ecl = cl[ed]
    eorder = np.argsort(ecl, kind="stable")
    es_s, ed_s = es[eorder], ed[eorder]
    cstarts = np.searchsorted(ecl[eorder], np.arange(C + 1))

    clusters = {}  # c -> (dst_u, src_u, S [nd, ns])
    for c in range(C):
        a, b = cstarts[c], cstarts[c + 1]
        eds, ess = ed_s[a:b], es_s[a:b]
        dst_u, di = np.unique(eds, return_inverse=True)
        src_u, si = np.unique(ess, return_inverse=True)
        S = np.zeros((dst_u.size, src_u.size), dtype=np.float32)
        # compensate prescale of dst-block rows?  No: sources come from
        # xts (unscaled copies), plain norm applies.
        np.add.at(S, (di, si), dinv[eds] * dinv[ess])
        clusters[c] = (dst_u, src_u, S)

    # within each device sort clusters by workload desc -> slots
    dev_clusters = [[] for _ in range(M)]
    for c in range(C):
        dev_clusters[cdev[c]].append(c)
    for d in range(M):
        dev_clusters[d].sort(
            key=lambda c: -(clusters[c][0].size + clusters[c][1].size)
        )

    ndcap = [0] * 8
    nscap = [0] * 8
    for j in range(8):
        ndcap[j] = max(clusters[dev_clusters[d][j]][0].size for d in range(M))
        nsmax = max(clusters[dev_clusters[d][j]][1].size for d in range(M))
        nscap[j] = _ceil_to(nsmax, 128) if nsmax else 0
    assert max(ndcap) <= 512 and max(nscap) <= 512, (ndcap, nscap)

    DSTR = int(sum(ndcap))
    xoff = np.concatenate([[0], np.cumsum([2 * s for s in nscap])]).astype(int)
    soff = np.concatenate(
        [[0], np.cumsum([(nscap[j] // 128) * ndcap[j] for j in range(8)])]
    ).astype(int)
    xsoff = np.concatenate(
        [[0], np.cumsum([(nscap[j] // 128) * D for j in range(8)])]
    ).astype(int)

    # plain nodes: all device nodes that are not a dst of their cluster
    is_dst = np.zeros(N, dtype=bool)
    for c in range(C):
        is_dst[clusters[c][0]] = True
    node_dev = cdev[cl]
    plain_lists = [
        np.where((node_dev == d) & ~is_dst)[0] for d in range(M)
    ]
    max_plain = max(p.size for p in plain_lists)
    MAIN_ROWS = DSTR + _ceil_to(max(max_plain, 1), 512)

    # split the dst region at a cluster boundary so the first dst
    # matmuls only wait for half the region's load
    half = int(sum(ndcap[:4]))
    chunks = [(0, half), (half, DSTR - half)] if 0 < half < DSTR else [(0, DSTR)]
    r0 = DSTR
    while r0 < MAIN_ROWS:
        L = min(_CHUNK, MAIN_ROWS - r0)
        chunks.append((r0, L))
        r0 += L

    w_pack = np.ascontiguousarray(
        weight.reshape(2, 128, D).transpose(1, 0, 2).astype(bf)
    )
    biasf = np.ascontiguousarray(bias.reshape(2, 128).T.astype(np.float32))

    in_maps = []
    main_ids = []
    for d in range(M):
        rows = np.zeros((MAIN_ROWS, D), dtype=np.float32)
        mid = np.full(MAIN_ROWS, -1, dtype=np.int64)
        xts = np.zeros((128, int(xoff[8])), dtype=bf)
        st = np.zeros((128, int(soff[8])), dtype=bf)
        pos = 0
        for j in range(8):
            c = dev_clusters[d][j]
            dst_u, src_u, S = clusters[c]
            nd, ns = dst_u.size, src_u.size
            rows[pos : pos + nd] = X[dst_u] * (dinv[dst_u] ** 2)[:, None]
            mid[pos : pos + nd] = dst_u
            pos += ndcap[j]
            if nscap[j]:
                xs = np.zeros((nscap[j], D), dtype=np.float32)
                xs[:ns] = X[src_u]
                xts[:, xoff[j] : xoff[j + 1]] = (
                    xs.reshape(nscap[j], 2, 128)
                    .transpose(2, 1, 0)
                    .reshape(128, 2 * nscap[j])
                )
                Sp = np.zeros((nscap[j], ndcap[j]), dtype=np.float32)
                Sp[:ns, :nd] = S.T
                st[:, soff[j] : soff[j + 1]] = (
                    Sp.reshape(nscap[j] // 128, 128, ndcap[j])
                    .transpose(1, 0, 2)
                    .reshape(128, (nscap[j] // 128) * ndcap[j])
                )
        plain = plain_lists[d]
        rows[DSTR : DSTR + plain.size] = X[plain]
        mid[DSTR : DSTR + plain.size] = plain
        main_ids.append(mid)
        in_maps.append({
            "xtm": _pack_rows(np.ascontiguousarray(rows.astype(bf)), chunks),
            "xts": np.ascontiguousarray(xts.astype(bf)),
            "st": np.ascontiguousarray(st.astype(bf)),
            "w": w_pack,
            "biasf": biasf,
        })

    nc = _build_program(
        ndcap, nscap, MAIN_ROWS, chunks, xoff, soff, xsoff
    )
    results = _run_program(nc, in_maps)

    epc = np.bincount(cl[ed], minlength=C)
    active = epc[cl] > 0

    out = X.copy()
    for d in range(M):
        rows = _unpack_rows(np.asarray(results[d]["outt"]), chunks, MAIN_ROWS)
        mid = main_ids[d]
        sel = mid >= 0
        ids = mid[sel]
        act = active[ids]
        out[ids[act]] = rows[sel][act]
    return out


# revision 12
# speedup vs baseline: 1.0867x; 1.0534x over previous
"""ClusterGCN on 8 trn2 cores — dense-S formulation, tight per-slot caps.

v2 over v1: per-cluster-slot capacity (ndcap_j exact, nscap_j 128-mult)
instead of one rectangular cap, src-only nodes emitted through the plain
stream (no separate outs tensor), flat-packed xts/st.  Cuts per-device
DMA from ~22.7MB to ~15.4MB.
"""

import numpy as np

N = 100000
D = 256
C = 64
M = 8  # cores

_CHUNK = 2048  # plain-region rows per load/store DMA (1MB bf16)


def _build_program(ndcap, nscap, MAIN_ROWS, chunks, xoff, soff, xsoff):
    import concourse.bacc as bacc
    import concourse.mybir as mybir
    from concourse import tile

    f32 = mybir.dt.float32
    bf16 = mybir.dt.bfloat16
    add = mybir.AluOpType.add
    DSTR = int(sum(ndcap))
    XTS_COLS = int(xoff[8])
    ST_COLS = int(soff[8])
    XS_COLS = int(xsoff[8])
    nd_max = int(max(max(ndcap), 1))
    nsk_max = int(max(max(c // 128 for c in nscap), 1))

    nc = bacc.Bacc("TRN2", target_bir_lowering=False, debug=False, num_devices=M)

    XTM = nc.dram_tensor("xtm", [128, 2 * MAIN_ROWS], bf16, kind="ExternalInput")
    XTS = nc.dram_tensor("xts", [128, XTS_COLS], bf16, kind="ExternalInput")
    ST = nc.dram_tensor("st", [128, ST_COLS], bf16, kind="ExternalInput")
    W = nc.dram_tensor("w", [128, 2, D], bf16, kind="ExternalInput")
    BF = nc.dram_tensor("biasf", [128, 2], f32, kind="ExternalInput")
    OUTT = nc.dram_tensor("outt", [128, 2 * MAIN_ROWS], bf16, kind="ExternalOutput")

    with tile.TileContext(nc) as tc:
        with (
            tc.tile_pool(name="const", bufs=1) as cpool,
            tc.tile_pool(name="xtm", bufs=len(chunks)) as xpool,
            tc.tile_pool(name="outt", bufs=4) as opool,
            tc.tile_pool(name="ps", bufs=4, space="PSUM") as ppool,
        ):
            # all loads on the sync HWDGE ring in dependency order (the
            # scalar ring starts ~3us late behind the ACT-table preamble);
            # stores go on the scalar ring.
            w_sb = cpool.tile([128, 2, D], bf16)
            nc.sync.dma_start(w_sb[:], W[:])
            bf_sb = cpool.tile([128, 2], f32)
            nc.sync.dma_start(bf_sb[:], BF[:])
            xts_sb = cpool.tile([128, XTS_COLS], bf16)
            nc.sync.dma_start(xts_sb[:], XTS[:])
            st_sb = cpool.tile([128, ST_COLS], bf16)

            xt_tiles = []
            for idx, (r0, L) in enumerate(chunks):
                xt = xpool.tile([128, 2 * L], bf16, tag="xt")
                nc.sync.dma_start(xt[:], XTM[:, 2 * r0 : 2 * r0 + 2 * L])
                xt_tiles.append(xt)
                if idx == 0:
                    # S^T right after the first dst half-chunk
                    nc.sync.dma_start(st_sb[:], ST[:])

            xsrc_sb = cpool.tile([128, XS_COLS], bf16)

            # PE warmup: ~3.4us of dummy matmuls on the (tiny, already
            # loaded) weight tile so the HAM clock-gate releases to
            # 2.4GHz before the real stream begins.  One accumulation
            # group into a scratch bank that is never read.
            pw = ppool.tile([128, 512], f32, tag="ps")
            for i in range(26):
                nc.tensor.matmul(
                    pw[:, :D],
                    w_sb[:, 0, 0:128],
                    w_sb[:, i % 2, :],
                    start=(i == 0),
                    stop=(i == 25),
                )

            # ---- src pass: Xl_src = X_src @ W, X-stationary ----
            for j in range(8):
                nsk = nscap[j] // 128
                if nsk == 0:
                    continue
                ps = ppool.tile([128, nsk_max, D], f32, tag="ps")
                for sk in range(nsk):
                    for k in range(2):
                        nc.tensor.matmul(
                            ps[:, sk, :],
                            xts_sb[
                                :,
                                xoff[j] + k * nscap[j] + sk * 128 :
                                xoff[j] + k * nscap[j] + (sk + 1) * 128,
                            ],
                            w_sb[:, k, :],
                            start=(k == 0),
                            stop=(k == 1),
                        )
                xv = xsrc_sb[:, xsoff[j] : xsoff[j] + nsk * D]
                pv = ps[:, :nsk, :]
                if j % 3 == 2:
                    nc.scalar.copy(xv, pv)
                else:
                    nc.vector.tensor_copy(xv, pv)

            eng = 0

            def epi(dst_ap, ps_ap, ft):
                nonlocal eng
                eng += 1
                if eng % 3 == 0:
                    nc.scalar.add(dst_ap, ps_ap, bf_sb[:, ft : ft + 1])
                else:
                    nc.vector.tensor_scalar(
                        dst_ap, ps_ap, bf_sb[:, ft : ft + 1], None, add
                    )

            # ---- main stream ----
            for ci, (r0, L) in enumerate(chunks):
                xt = xt_tiles[ci]
                ot = opool.tile([128, 2 * L], bf16, tag="ot")
                pos = r0
                while pos < r0 + L:
                    off = pos - r0
                    if pos < DSTR:
                        j = next(
                            jj for jj in range(8)
                            if sum(ndcap[:jj]) == pos
                        )
                        nd = ndcap[j]
                        nsk = nscap[j] // 128
                        # each ft slice must be bank-aligned: a matmul
                        # output AP may not cross a 2KB PSUM bank boundary
                        psd = ppool.tile([128, 2, 512], f32, tag="ps")
                        for ft in range(2):
                            for k in range(2):
                                nc.tensor.matmul(
                                    psd[:, ft, :nd],
                                    w_sb[:, k, ft * 128 : (ft + 1) * 128],
                                    xt[:, k * L + off : k * L + off + nd],
                                    start=(k == 0),
                                    stop=(nsk == 0 and k == 1),
                                )
                            for sk in range(nsk):
                                nc.tensor.matmul(
                                    psd[:, ft, :nd],
                                    xsrc_sb[
                                        :,
                                        xsoff[j] + sk * D + ft * 128 :
                                        xsoff[j] + sk * D + (ft + 1) * 128,
                                    ],
                                    st_sb[
                                        :,
                                        soff[j] + sk * nd : soff[j] + (sk + 1) * nd,
                                    ],
                                    start=False,
                                    stop=(sk == nsk - 1),
                                )
                            epi(
                                ot[:, ft * L + off : ft * L + off + nd],
                                psd[:, ft, :nd],
                                ft,
                            )
                        pos += nd
                    else:
                        n = min(512, r0 + L - pos)
                        ps = ppool.tile([128, 2, 512], f32, tag="ps")
                        for ft in range(2):
                            for k in range(2):
                                nc.tensor.matmul(
                                    ps[:, ft, :n],
                                    w_sb[:, k, ft * 128 : (ft + 1) * 128],
                                    xt[:, k * L + off : k * L + off + n],
                                    start=(k == 0),
                                    stop=(k == 1),
                                )
                            epi(
                                ot[:, ft * L + off : ft * L + off + n],
                                ps[:, ft, :n],
                                ft,
                            )
                        pos += n
                nc.scalar.dma_start(OUTT[:, 2 * r0 : 2 * r0 + 2 * L], ot[:])

    nc.compile()
    return nc


def _run_program(nc, in_maps):
    from concourse.bass_utils import run_bass_kernel_spmd

    return run_bass_kernel_spmd(nc, in_maps, core_ids=list(range(M))).results


def _ceil_to(x, m):
    return -(-x // m) * m


def _pack_rows(rows_bf16, chunks):
    out = np.empty((128, 2 * rows_bf16.shape[0]), dtype=rows_bf16.dtype)
    for r0, L in chunks:
        seg = rows_bf16[r0 : r0 + L]
        out[:, 2 * r0 : 2 * r0 + 2 * L] = (
            seg.reshape(L, 2, 128).transpose(2, 1, 0).reshape(128, 2 * L)
        )
    return out


def _unpack_rows(packed, chunks, main_rows):
    rows = np.empty((main_rows, 256), dtype=np.float32)
    for r0, L in chunks:
        blk = packed[:, 2 * r0 : 2 * r0 + 2 * L].reshape(128, 2, L)
        rows[r0 : r0 + L] = (
            blk.transpose(2, 1, 0).reshape(L, 256).astype(np.float32)
        )
    return rows


def kernel(X, weight, bias, cluster_assignment, edge_index):
    import ml_dtypes

    bf = ml_dtypes.bfloat16
    X = np.ascontiguousarray(np.asarray(X, dtype=np.float32))
    weight = np.ascontiguousarray(np.asarray(weight, dtype=np.float32))
    bias = np.asarray(bias, dtype=np.float32)
    cl = np.asarray(cluster_assignment).astype(np.int64)
    ei = np.asarray(edge_index).astype(np.int64)

    src, dst = ei[0], ei[1]
    intra = cl[src] == cl[dst]
    es, ed = src[intra], dst[intra]

    deg = (np.bincount(ed, minlength=N) + 1.0).astype(np.float32)
    dinv = (1.0 / np.sqrt(deg)).astype(np.float32)

    # clusters -> devices: snake over size-sorted clusters, 8 per device
    csize = np.bincount(cl, minlength=C)
    order = np.argsort(-csize, kind="stable")
    cdev = np.zeros(C, dtype=np.int64)
    for i, c in enumerate(order):
        r, q = divmod(i, M)
        cdev[c] = q if r % 2 == 0 else M - 1 - q

    # group intra edges by cluster
    ecl = cl[ed]
    eorder = np.argsort(ecl, kind="stable")
    es_s, ed_s = es[eorder], ed[eorder]
    cstarts = np.searchsorted(ecl[eorder], np.arange(C + 1))

    clusters = {}  # c -> (dst_u, src_u, S [nd, ns])
    for c in range(C):
        a, b = cstarts[c], cstarts[c + 1]
        eds, ess = ed_s[a:b], es_s[a:b]
        dst_u, di = np.unique(eds, return_inverse=True)
        src_u, si = np.unique(ess, return_inverse=True)
        S = np.zeros((dst_u.size, src_u.size), dtype=np.float32)
        # compensate prescale of dst-block rows?  No: sources come from
        # xts (unscaled copies), plain norm applies.
        np.add.at(S, (di, si), dinv[eds] * dinv[ess])
        clusters[c] = (dst_u, src_u, S)

    # within each device sort clusters by workload desc -> slots
    dev_clusters = [[] for _ in range(M)]
    for c in range(C):
        dev_clusters[cdev[c]].append(c)
    for d in range(M):
        dev_clusters[d].sort(
            key=lambda c: -(clusters[c][0].size + clusters[c][1].size)
        )

    ndcap = [0] * 8
    nscap = [0] * 8
    for j in range(8):
        ndcap[j] = max(clusters[dev_clusters[d][j]][0].size for d in range(M))
        nsmax = max(clusters[dev_clusters[d][j]][1].size for d in range(M))
        nscap[j] = _ceil_to(nsmax, 128) if nsmax else 0
    assert max(ndcap) <= 512 and max(nscap) <= 512, (ndcap, nscap)

    DSTR = int(sum(ndcap))
    xoff = np.concatenate([[0], np.cumsum([2 * s for s in nscap])]).astype(int)
    soff = np.concatenate(
        [[0], np.cumsum([(nscap[j] // 128) * ndcap[j] for j in range(8)])]
    ).astype(int)
    xsoff = np.concatenate(
        [[0], np.cumsum([(nscap[j] // 128) * D for j in range(8)])]
    ).astype(int)

    # plain nodes: all device nodes that are not a dst of their cluster
    is_dst = np.zeros(N, dtype=bool)
    for c in range(C):
        is_dst[clusters[c][0]] = True
    node_dev = cdev[cl]
    plain_lists = [
        np.where((node_dev == d) & ~is_dst)[0] for d in range(M)
    ]
    max_plain = max(p.size for p in plain_lists)
    MAIN_ROWS = DSTR + _ceil_to(max(max_plain, 1), 512)

    # split the dst region at a cluster boundary so the first dst
    # matmuls only wait for half the region's load
    half = int(sum(ndcap[:4]))
    chunks = [(0, half), (half, DSTR - half)] if 0 < half < DSTR else [(0, DSTR)]
    r0 = DSTR
    while r0 < MAIN_ROWS:
        L = min(_CHUNK, MAIN_ROWS - r0)
        chunks.append((r0, L))
        r0 += L

    w_pack = np.ascontiguousarray(
        weight.reshape(2, 128, D).transpose(1, 0, 2).astype(bf)
    )
    biasf = np.ascontiguousarray(bias.reshape(2, 128).T.astype(np.float32))

    in_maps = []
    main_ids = []
    for d in range(M):
        rows = np.zeros((MAIN_ROWS, D), dtype=np.float32)
        mid = np.full(MAIN_ROWS, -1, dtype=np.int64)
        xts = np.zeros((128, int(xoff[8])), dtype=bf)
        st = np.zeros((128, int(soff[8])), dtype=bf)
        pos = 0
        for j in range(8):
            c = dev_clusters[d][j]
            dst_u, src_u, S = clusters[c]
            nd, ns = dst_u.size, src_u.size
            rows[pos : pos + nd] = X[dst_u] * (dinv[dst_u] ** 2)[:, None]
            mid[pos : pos + nd] = dst_u
            pos += ndcap[j]
            if nscap[j]:
                xs = np.zeros((nscap[j], D), dtype=np.float32)
                xs[:ns] = X[src_u]
                xts[:, xoff[j] : xoff[j + 1]] = (
                    xs.reshape(nscap[j], 2, 128)
                    .transpose(2, 1, 0)
                    .reshape(128, 2 * nscap[j])
                )
                Sp = np.zeros((nscap[j], ndcap[j]), dtype=np.float32)
                Sp[:ns, :nd] = S.T
                st[:, soff[j] : soff[j + 1]] = (
                    Sp.reshape(nscap[j] // 128, 128, ndcap[j])
                    .transpose(1, 0, 2)
                    .reshape(128, (nscap[j] // 128) * ndcap[j])
                )
        plain = plain_lists[d]
        rows[DSTR : DSTR + plain.size] = X[plain]
        mid[DSTR : DSTR + plain.size] = plain
        main_ids.append(mid)
        in_maps.append({
            "xtm": _pack_rows(np.ascontiguousarray(rows.astype(bf)), chunks),
            "xts": np.ascontiguousarray(xts.astype(bf)),
            "st": np.ascontiguousarray(st.astype(bf)),
            "w": w_pack,
            "biasf": biasf,
        })

    nc = _build_program(
        ndcap, nscap, MAIN_ROWS, chunks, xoff, soff, xsoff
    )
    results = _run_program(nc, in_maps)

    epc = np.bincount(cl[ed], minlength=C)
    active = epc[cl] > 0

    out = X.copy()
    for d in range(M):
        rows = _unpack_rows(np.asarray(results[d]["outt"]), chunks, MAIN_ROWS)
        mid = main_ids[d]
        sel = mid >= 0
        ids = mid[sel]
        act = active[ids]
        out[ids[act]] = rows[sel][act]
    return out


# revision 16
# speedup vs baseline: 1.1450x; 1.0537x over previous
"""ClusterGCN on 8 trn2 cores — dense-S formulation, tight per-slot caps.

v2 over v1: per-cluster-slot capacity (ndcap_j exact, nscap_j 128-mult)
instead of one rectangular cap, src-only nodes emitted through the plain
stream (no separate outs tensor), flat-packed xts/st.  Cuts per-device
DMA from ~22.7MB to ~15.4MB.
"""

import numpy as np

N = 100000
D = 256
C = 64
M = 8  # cores

_CHUNK = 2048  # plain-region rows per load/store DMA (1MB bf16)


def _build_program(ndcap, nscap, MAIN_ROWS, chunks, xoff, soff, xsoff):
    import concourse.bacc as bacc
    import concourse.mybir as mybir
    from concourse import tile

    f32 = mybir.dt.float32
    bf16 = mybir.dt.bfloat16
    add = mybir.AluOpType.add
    DSTR = int(sum(ndcap))
    XTS_COLS = int(xoff[8])
    ST_COLS = int(soff[8])
    XS_COLS = int(xsoff[8])
    nd_max = int(max(max(ndcap), 1))
    nsk_max = int(max(max(c // 128 for c in nscap), 1))

    nc = bacc.Bacc("TRN2", target_bir_lowering=False, debug=False, num_devices=M)

    XTM = nc.dram_tensor("xtm", [128, 2 * MAIN_ROWS], bf16, kind="ExternalInput")
    XTS = nc.dram_tensor("xts", [128, XTS_COLS], bf16, kind="ExternalInput")
    ST = nc.dram_tensor("st", [128, ST_COLS], bf16, kind="ExternalInput")
    W = nc.dram_tensor("w", [128, 2, D], bf16, kind="ExternalInput")
    BF = nc.dram_tensor("biasf", [128, 2], f32, kind="ExternalInput")
    OUTT = nc.dram_tensor("outt", [128, 2 * MAIN_ROWS], bf16, kind="ExternalOutput")

    with tile.TileContext(nc) as tc:
        with (
            tc.tile_pool(name="const", bufs=1) as cpool,
            tc.tile_pool(name="xtm", bufs=len(chunks)) as xpool,
            tc.tile_pool(name="outt", bufs=4) as opool,
            tc.tile_pool(name="ps", bufs=4, space="PSUM") as ppool,
        ):
            # all loads on the sync HWDGE ring in dependency order (the
            # scalar ring starts ~3us late behind the ACT-table preamble);
            # stores go on the scalar ring.
            w_sb = cpool.tile([128, 2, D], bf16)
            nc.sync.dma_start(w_sb[:], W[:])
            bf_sb = cpool.tile([128, 2], f32)
            nc.sync.dma_start(bf_sb[:], BF[:])
            xts_sb = cpool.tile([128, XTS_COLS], bf16)
            nc.sync.dma_start(xts_sb[:], XTS[:])
            st_sb = cpool.tile([128, ST_COLS], bf16)

            xt_tiles = []
            for idx, (r0, L) in enumerate(chunks):
                xt = xpool.tile([128, 2 * L], bf16, tag="xt")
                nc.sync.dma_start(xt[:], XTM[:, 2 * r0 : 2 * r0 + 2 * L])
                xt_tiles.append(xt)
                if idx == 0:
                    # S^T right after the first dst half-chunk
                    nc.sync.dma_start(st_sb[:], ST[:])

            xsrc_sb = cpool.tile([128, XS_COLS], bf16)

            # PE warmup: ~3.4us of dummy matmuls on the (tiny, already
            # loaded) weight tile so the HAM clock-gate releases to
            # 2.4GHz before the real stream begins.  One accumulation
            # group into a scratch bank that is never read.
            pw = ppool.tile([128, 512], f32, tag="ps")
            for i in range(15):
                nc.tensor.matmul(
                    pw[:, :D],
                    w_sb[:, 0, 0:128],
                    w_sb[:, i % 2, :],
                    start=(i == 0),
                    stop=(i == 14),
                )

            # ---- src pass: Xl_src = X_src @ W, X-stationary ----
            for j in range(8):
                nsk = nscap[j] // 128
                if nsk == 0:
                    continue
                ps = ppool.tile([128, nsk_max, D], f32, tag="ps")
                for sk in range(nsk):
                    for k in range(2):
                        nc.tensor.matmul(
                            ps[:, sk, :],
                            xts_sb[
                                :,
                                xoff[j] + k * nscap[j] + sk * 128 :
                                xoff[j] + k * nscap[j] + (sk + 1) * 128,
                            ],
                            w_sb[:, k, :],
                            start=(k == 0),
                            stop=(k == 1),
                        )
                xv = xsrc_sb[:, xsoff[j] : xsoff[j] + nsk * D]
                pv = ps[:, :nsk, :]
                if j % 2 == 0:
                    nc.scalar.copy(xv, pv)
                else:
                    nc.vector.tensor_copy(xv, pv)

            eng = 0

            def epi(dst_ap, ps_ap, ft):
                nonlocal eng
                eng += 1
                if eng % 2 == 0:
                    nc.scalar.add(dst_ap, ps_ap, bf_sb[:, ft : ft + 1])
                else:
                    nc.vector.tensor_scalar(
                        dst_ap, ps_ap, bf_sb[:, ft : ft + 1], None, add
                    )

            # ---- main stream ----
            for ci, (r0, L) in enumerate(chunks):
                xt = xt_tiles[ci]
                ot = opool.tile([128, 2 * L], bf16, tag="ot")
                pos = r0
                while pos < r0 + L:
                    off = pos - r0
                    if pos < DSTR:
                        j = next(
                            jj for jj in range(8)
                            if sum(ndcap[:jj]) == pos
                        )
                        nd = ndcap[j]
                        nsk = nscap[j] // 128
                        # each ft slice must be bank-aligned: a matmul
                        # output AP may not cross a 2KB PSUM bank boundary
                        psd = ppool.tile([128, 2, 512], f32, tag="ps")
                        for ft in range(2):
                            for k in range(2):
                                nc.tensor.matmul(
                                    psd[:, ft, :nd],
                                    w_sb[:, k, ft * 128 : (ft + 1) * 128],
                                    xt[:, k * L + off : k * L + off + nd],
                                    start=(k == 0),
                                    stop=(nsk == 0 and k == 1),
                                )
                            for sk in range(nsk):
                                nc.tensor.matmul(
                                    psd[:, ft, :nd],
                                    xsrc_sb[
                                        :,
                                        xsoff[j] + sk * D + ft * 128 :
                                        xsoff[j] + sk * D + (ft + 1) * 128,
                                    ],
                                    st_sb[
                                        :,
                                        soff[j] + sk * nd : soff[j] + (sk + 1) * nd,
                                    ],
                                    start=False,
                                    stop=(sk == nsk - 1),
                                )
                            epi(
                                ot[:, ft * L + off : ft * L + off + nd],
                                psd[:, ft, :nd],
                                ft,
                            )
                        pos += nd
                    else:
                        n = min(512, r0 + L - pos)
                        ps = ppool.tile([128, 2, 512], f32, tag="ps")
                        for ft in range(2):
                            for k in range(2):
                                nc.tensor.matmul(
                                    ps[:, ft, :n],
                                    w_sb[:, k, ft * 128 : (ft + 1) * 128],
                                    xt[:, k * L + off : k * L + off + n],
                                    start=(k == 0),
                                    stop=(k == 1),
                                )
                            epi(
                                ot[:, ft * L + off : ft * L + off + n],
                                ps[:, ft, :n],
                                ft,
                            )
                        pos += n
                nc.scalar.dma_start(OUTT[:, 2 * r0 : 2 * r0 + 2 * L], ot[:])

    nc.compile()
    return nc


def _run_program(nc, in_maps):
    from concourse.bass_utils import run_bass_kernel_spmd

    return run_bass_kernel_spmd(nc, in_maps, core_ids=list(range(M))).results


def _ceil_to(x, m):
    return -(-x // m) * m


def _pack_rows(rows_bf16, chunks):
    out = np.empty((128, 2 * rows_bf16.shape[0]), dtype=rows_bf16.dtype)
    for r0, L in chunks:
        seg = rows_bf16[r0 : r0 + L]
        out[:, 2 * r0 : 2 * r0 + 2 * L] = (
            seg.reshape(L, 2, 128).transpose(2, 1, 0).reshape(128, 2 * L)
        )
    return out


def _unpack_rows(packed, chunks, main_rows):
    rows = np.empty((main_rows, 256), dtype=np.float32)
    for r0, L in chunks:
        blk = packed[:, 2 * r0 : 2 * r0 + 2 * L].reshape(128, 2, L)
        rows[r0 : r0 + L] = (
            blk.transpose(2, 1, 0).reshape(L, 256).astype(np.float32)
        )
    return rows


def kernel(X, weight, bias, cluster_assignment, edge_index):
    import ml_dtypes

    bf = ml_dtypes.bfloat16
    X = np.ascontiguousarray(np.asarray(X, dtype=np.float32))
    weight = np.ascontiguousarray(np.asarray(weight, dtype=np.float32))
    bias = np.asarray(bias, dtype=np.float32)
    cl = np.asarray(cluster_assignment).astype(np.int64)
    ei = np.asarray(edge_index).astype(np.int64)

    src, dst = ei[0], ei[1]
    intra = cl[src] == cl[dst]
    es, ed = src[intra], dst[intra]

    deg = (np.bincount(ed, minlength=N) + 1.0).astype(np.float32)
    dinv = (1.0 / np.sqrt(deg)).astype(np.float32)

    # clusters -> devices: snake over size-sorted clusters, 8 per device
    csize = np.bincount(cl, minlength=C)
    order = np.argsort(-csize, kind="stable")
    cdev = np.zeros(C, dtype=np.int64)
    for i, c in enumerate(order):
        r, q = divmod(i, M)
        cdev[c] = q if r % 2 == 0 else M - 1 - q

    # group intra edges by cluster
    ecl = cl[ed]
    eorder = np.argsort(ecl, kind="stable")
    es_s, ed_s = es[eorder], ed[eorder]
    cstarts = np.searchsorted(ecl[eorder], np.arange(C + 1))

    clusters = {}  # c -> (dst_u, src_u, S [nd, ns])
    for c in range(C):
        a, b = cstarts[c], cstarts[c + 1]
        eds, ess = ed_s[a:b], es_s[a:b]
        dst_u, di = np.unique(eds, return_inverse=True)
        src_u, si = np.unique(ess, return_inverse=True)
        S = np.zeros((dst_u.size, src_u.size), dtype=np.float32)
        # compensate prescale of dst-block rows?  No: sources come from
        # xts (unscaled copies), plain norm applies.
        np.add.at(S, (di, si), dinv[eds] * dinv[ess])
        clusters[c] = (dst_u, src_u, S)

    # within each device sort clusters by workload desc -> slots
    dev_clusters = [[] for _ in range(M)]
    for c in range(C):
        dev_clusters[cdev[c]].append(c)
    for d in range(M):
        dev_clusters[d].sort(
            key=lambda c: -(clusters[c][0].size + clusters[c][1].size)
        )

    ndcap = [0] * 8
    nscap = [0] * 8
    for j in range(8):
        ndcap[j] = max(clusters[dev_clusters[d][j]][0].size for d in range(M))
        nsmax = max(clusters[dev_clusters[d][j]][1].size for d in range(M))
        nscap[j] = _ceil_to(nsmax, 128) if nsmax else 0
    assert max(ndcap) <= 512 and max(nscap) <= 512, (ndcap, nscap)

    DSTR = int(sum(ndcap))
    xoff = np.concatenate([[0], np.cumsum([2 * s for s in nscap])]).astype(int)
    soff = np.concatenate(
        [[0], np.cumsum([(nscap[j] // 128) * ndcap[j] for j in range(8)])]
    ).astype(int)
    xsoff = np.concatenate(
        [[0], np.cumsum([(nscap[j] // 128) * D for j in range(8)])]
    ).astype(int)

    # plain nodes: all device nodes that are not a dst of their cluster
    is_dst = np.zeros(N, dtype=bool)
    for c in range(C):
        is_dst[clusters[c][0]] = True
    node_dev = cdev[cl]
    plain_lists = [
        np.where((node_dev == d) & ~is_dst)[0] for d in range(M)
    ]
    max_plain = max(p.size for p in plain_lists)
    MAIN_ROWS = DSTR + _ceil_to(max(max_plain, 1), 512)

    # split the dst region at a cluster boundary so the first dst
    # matmuls only wait for half the region's load
    half = int(sum(ndcap[:4]))
    chunks = [(0, half), (half, DSTR - half)] if 0 < half < DSTR else [(0, DSTR)]
    r0 = DSTR
    while r0 < MAIN_ROWS:
        # small chunks at the end so the last store doesn't trail far
        L = min(_CHUNK if MAIN_ROWS - r0 > 3072 else 1024, MAIN_ROWS - r0)
        chunks.append((r0, L))
        r0 += L

    w_pack = np.ascontiguousarray(
        weight.reshape(2, 128, D).transpose(1, 0, 2).astype(bf)
    )
    biasf = np.ascontiguousarray(bias.reshape(2, 128).T.astype(np.float32))

    in_maps = []
    main_ids = []
    for d in range(M):
        rows = np.zeros((MAIN_ROWS, D), dtype=np.float32)
        mid = np.full(MAIN_ROWS, -1, dtype=np.int64)
        xts = np.zeros((128, int(xoff[8])), dtype=bf)
        st = np.zeros((128, int(soff[8])), dtype=bf)
        pos = 0
        for j in range(8):
            c = dev_clusters[d][j]
            dst_u, src_u, S = clusters[c]
            nd, ns = dst_u.size, src_u.size
            rows[pos : pos + nd] = X[dst_u] * (dinv[dst_u] ** 2)[:, None]
            mid[pos : pos + nd] = dst_u
            pos += ndcap[j]
            if nscap[j]:
                xs = np.zeros((nscap[j], D), dtype=np.float32)
                xs[:ns] = X[src_u]
                xts[:, xoff[j] : xoff[j + 1]] = (
                    xs.reshape(nscap[j], 2, 128)
                    .transpose(2, 1, 0)
                    .reshape(128, 2 * nscap[j])
                )
                Sp = np.zeros((nscap[j], ndcap[j]), dtype=np.float32)
                Sp[:ns, :nd] = S.T
                st[:, soff[j] : soff[j + 1]] = (
                    Sp.reshape(nscap[j] // 128, 128, ndcap[j])
                    .transpose(1, 0, 2)
                    .reshape(128, (nscap[j] // 128) * ndcap[j])
                )
        plain = plain_lists[d]
        rows[DSTR : DSTR + plain.size] = X[plain]
        mid[DSTR : DSTR + plain.size] = plain
        main_ids.append(mid)
        in_maps.append({
            "xtm": _pack_rows(np.ascontiguousarray(rows.astype(bf)), chunks),
            "xts": np.ascontiguousarray(xts.astype(bf)),
            "st": np.ascontiguousarray(st.astype(bf)),
            "w": w_pack,
            "biasf": biasf,
        })

    nc = _build_program(
        ndcap, nscap, MAIN_ROWS, chunks, xoff, soff, xsoff
    )
    results = _run_program(nc, in_maps)

    epc = np.bincount(cl[ed], minlength=C)
    active = epc[cl] > 0

    out = X.copy()
    for d in range(M):
        rows = _unpack_rows(np.asarray(results[d]["outt"]), chunks, MAIN_ROWS)
        mid = main_ids[d]
        sel = mid >= 0
        ids = mid[sel]
        act = active[ids]
        out[ids[act]] = rows[sel][act]
    return out


# revision 17
# speedup vs baseline: 1.1903x; 1.0395x over previous
"""ClusterGCN on 8 trn2 cores — dense-S formulation, tight per-slot caps.

v2 over v1: per-cluster-slot capacity (ndcap_j exact, nscap_j 128-mult)
instead of one rectangular cap, src-only nodes emitted through the plain
stream (no separate outs tensor), flat-packed xts/st.  Cuts per-device
DMA from ~22.7MB to ~15.4MB.
"""

import numpy as np

N = 100000
D = 256
C = 64
M = 8  # cores

_CHUNK = 2048  # plain-region rows per load/store DMA (1MB bf16)


def _build_program(ndcap, nscap, MAIN_ROWS, chunks, xoff, soff, xsoff):
    import concourse.bacc as bacc
    import concourse.mybir as mybir
    from concourse import tile

    f32 = mybir.dt.float32
    bf16 = mybir.dt.bfloat16
    add = mybir.AluOpType.add
    DSTR = int(sum(ndcap))
    XTS_COLS = int(xoff[8])
    ST_COLS = int(soff[8])
    XS_COLS = int(xsoff[8])
    nd_max = int(max(max(ndcap), 1))
    nsk_max = int(max(max(c // 128 for c in nscap), 1))

    nc = bacc.Bacc("TRN2", target_bir_lowering=False, debug=False, num_devices=M)

    XTM = nc.dram_tensor("xtm", [128, 2 * MAIN_ROWS], bf16, kind="ExternalInput")
    XTS = nc.dram_tensor("xts", [128, XTS_COLS], bf16, kind="ExternalInput")
    ST = nc.dram_tensor("st", [128, ST_COLS], bf16, kind="ExternalInput")
    W = nc.dram_tensor("w", [128, 2, D], bf16, kind="ExternalInput")
    BF = nc.dram_tensor("biasf", [128, 2], f32, kind="ExternalInput")
    OUTT = nc.dram_tensor("outt", [128, 2 * MAIN_ROWS], bf16, kind="ExternalOutput")

    with tile.TileContext(nc) as tc:
        with (
            tc.tile_pool(name="const", bufs=1) as cpool,
            tc.tile_pool(name="xtm", bufs=len(chunks)) as xpool,
            tc.tile_pool(name="outt", bufs=4) as opool,
            tc.tile_pool(name="ps", bufs=4, space="PSUM") as ppool,
        ):
            # all loads on the sync HWDGE ring in dependency order (the
            # scalar ring starts ~3us late behind the ACT-table preamble);
            # stores go on the scalar ring.
            w_sb = cpool.tile([128, 2, D], bf16)
            nc.sync.dma_start(w_sb[:], W[:])
            bf_sb = cpool.tile([128, 2], f32)
            nc.sync.dma_start(bf_sb[:], BF[:])
            xts_sb = cpool.tile([128, XTS_COLS], bf16)
            nc.sync.dma_start(xts_sb[:], XTS[:])
            st_sb = cpool.tile([128, ST_COLS], bf16)

            xt_tiles = []
            for idx, (r0, L) in enumerate(chunks):
                xt = xpool.tile([128, 2 * L], bf16, tag="xt")
                nc.sync.dma_start(xt[:], XTM[:, 2 * r0 : 2 * r0 + 2 * L])
                xt_tiles.append(xt)
                if idx == 0:
                    # S^T right after the first dst half-chunk
                    nc.sync.dma_start(st_sb[:], ST[:])

            xsrc_sb = cpool.tile([128, XS_COLS], bf16)

            # PE warmup: ~3.4us of dummy matmuls on the (tiny, already
            # loaded) weight tile so the HAM clock-gate releases to
            # 2.4GHz before the real stream begins.  One accumulation
            # group into a scratch bank that is never read.
            pw = ppool.tile([128, 512], f32, tag="ps")
            for i in range(24):
                nc.tensor.matmul(
                    pw[:, :D],
                    w_sb[:, 0, 0:128],
                    w_sb[:, i % 2, :],
                    start=(i == 0),
                    stop=(i == 23),
                )

            # ---- src pass: Xl_src = X_src @ W, X-stationary ----
            for j in range(8):
                nsk = nscap[j] // 128
                if nsk == 0:
                    continue
                ps = ppool.tile([128, nsk_max, D], f32, tag="ps")
                for sk in range(nsk):
                    for k in range(2):
                        nc.tensor.matmul(
                            ps[:, sk, :],
                            xts_sb[
                                :,
                                xoff[j] + k * nscap[j] + sk * 128 :
                                xoff[j] + k * nscap[j] + (sk + 1) * 128,
                            ],
                            w_sb[:, k, :],
                            start=(k == 0),
                            stop=(k == 1),
                        )
                xv = xsrc_sb[:, xsoff[j] : xsoff[j] + nsk * D]
                pv = ps[:, :nsk, :]
                if j % 2 == 0:
                    nc.scalar.copy(xv, pv)
                else:
                    nc.vector.tensor_copy(xv, pv)

            eng = 0

            def epi(dst_ap, ps_ap, ft):
                nonlocal eng
                eng += 1
                if eng % 2 == 0:
                    nc.scalar.add(dst_ap, ps_ap, bf_sb[:, ft : ft + 1])
                else:
                    nc.vector.tensor_scalar(
                        dst_ap, ps_ap, bf_sb[:, ft : ft + 1], None, add
                    )

            # ---- main stream ----
            for ci, (r0, L) in enumerate(chunks):
                xt = xt_tiles[ci]
                ot = opool.tile([128, 2 * L], bf16, tag="ot")
                pos = r0
                while pos < r0 + L:
                    off = pos - r0
                    if pos < DSTR:
                        j = next(
                            jj for jj in range(8)
                            if sum(ndcap[:jj]) == pos
                        )
                        nd = ndcap[j]
                        nsk = nscap[j] // 128
                        # each ft slice must be bank-aligned: a matmul
                        # output AP may not cross a 2KB PSUM bank boundary
                        psd = ppool.tile([128, 2, 512], f32, tag="ps")
                        for ft in range(2):
                            for k in range(2):
                                nc.tensor.matmul(
                                    psd[:, ft, :nd],
                                    w_sb[:, k, ft * 128 : (ft + 1) * 128],
                                    xt[:, k * L + off : k * L + off + nd],
                                    start=(k == 0),
                                    stop=(nsk == 0 and k == 1),
                                )
                            for sk in range(nsk):
                                nc.tensor.matmul(
                                    psd[:, ft, :nd],
                                    xsrc_sb[
                                        :,
                                        xsoff[j] + sk * D + ft * 128 :
                                        xsoff[j] + sk * D + (ft + 1) * 128,
                                    ],
                                    st_sb[
                                        :,
                                        soff[j] + sk * nd : soff[j] + (sk + 1) * nd,
                                    ],
                                    start=False,
                                    stop=(sk == nsk - 1),
                                )
                            epi(
                                ot[:, ft * L + off : ft * L + off + nd],
                                psd[:, ft, :nd],
                                ft,
                            )
                        pos += nd
                    else:
                        n = min(512, r0 + L - pos)
                        ps = ppool.tile([128, 2, 512], f32, tag="ps")
                        for ft in range(2):
                            for k in range(2):
                                nc.tensor.matmul(
                                    ps[:, ft, :n],
                                    w_sb[:, k, ft * 128 : (ft + 1) * 128],
                                    xt[:, k * L + off : k * L + off + n],
                                    start=(k == 0),
                                    stop=(k == 1),
                                )
                            epi(
                                ot[:, ft * L + off : ft * L + off + n],
                                ps[:, ft, :n],
                                ft,
                            )
                        pos += n
                nc.scalar.dma_start(OUTT[:, 2 * r0 : 2 * r0 + 2 * L], ot[:])

    nc.compile()
    return nc


def _run_program(nc, in_maps):
    from concourse.bass_utils import run_bass_kernel_spmd

    return run_bass_kernel_spmd(nc, in_maps, core_ids=list(range(M))).results


def _ceil_to(x, m):
    return -(-x // m) * m


def _pack_rows(rows_bf16, chunks):
    out = np.empty((128, 2 * rows_bf16.shape[0]), dtype=rows_bf16.dtype)
    for r0, L in chunks:
        seg = rows_bf16[r0 : r0 + L]
        out[:, 2 * r0 : 2 * r0 + 2 * L] = (
            seg.reshape(L, 2, 128).transpose(2, 1, 0).reshape(128, 2 * L)
        )
    return out


def _unpack_rows(packed, chunks, main_rows):
    rows = np.empty((main_rows, 256), dtype=np.float32)
    for r0, L in chunks:
        blk = packed[:, 2 * r0 : 2 * r0 + 2 * L].reshape(128, 2, L)
        rows[r0 : r0 + L] = (
            blk.transpose(2, 1, 0).reshape(L, 256).astype(np.float32)
        )
    return rows


def kernel(X, weight, bias, cluster_assignment, edge_index):
    import ml_dtypes

    bf = ml_dtypes.bfloat16
    X = np.ascontiguousarray(np.asarray(X, dtype=np.float32))
    weight = np.ascontiguousarray(np.asarray(weight, dtype=np.float32))
    bias = np.asarray(bias, dtype=np.float32)
    cl = np.asarray(cluster_assignment).astype(np.int64)
    ei = np.asarray(edge_index).astype(np.int64)

    src, dst = ei[0], ei[1]
    intra = cl[src] == cl[dst]
    es, ed = src[intra], dst[intra]

    deg = (np.bincount(ed, minlength=N) + 1.0).astype(np.float32)
    dinv = (1.0 / np.sqrt(deg)).astype(np.float32)

    # clusters -> devices: snake over size-sorted clusters, 8 per device
    csize = np.bincount(cl, minlength=C)
    order = np.argsort(-csize, kind="stable")
    cdev = np.zeros(C, dtype=np.int64)
    for i, c in enumerate(order):
        r, q = divmod(i, M)
        cdev[c] = q if r % 2 == 0 else M - 1 - q

    # group intra edges by cluster
    ecl = cl[ed]
    eorder = np.argsort(ecl, kind="stable")
    es_s, ed_s = es[eorder], ed[eorder]
    cstarts = np.searchsorted(ecl[eorder], np.arange(C + 1))

    clusters = {}  # c -> (dst_u, src_u, S [nd, ns])
    for c in range(C):
        a, b = cstarts[c], cstarts[c + 1]
        eds, ess = ed_s[a:b], es_s[a:b]
        dst_u, di = np.unique(eds, return_inverse=True)
        src_u, si = np.unique(ess, return_inverse=True)
        S = np.zeros((dst_u.size, src_u.size), dtype=np.float32)
        # compensate prescale of dst-block rows?  No: sources come from
        # xts (unscaled copies), plain norm applies.
        np.add.at(S, (di, si), dinv[eds] * dinv[ess])
        clusters[c] = (dst_u, src_u, S)

    # within each device sort clusters by workload desc -> slots
    dev_clusters = [[] for _ in range(M)]
    for c in range(C):
        dev_clusters[cdev[c]].append(c)
    for d in range(M):
        dev_clusters[d].sort(
            key=lambda c: -(clusters[c][0].size + clusters[c][1].size)
        )

    ndcap = [0] * 8
    nscap = [0] * 8
    for j in range(8):
        ndcap[j] = max(clusters[dev_clusters[d][j]][0].size for d in range(M))
        nsmax = max(clusters[dev_clusters[d][j]][1].size for d in range(M))
        nscap[j] = _ceil_to(nsmax, 128) if nsmax else 0
    assert max(ndcap) <= 512 and max(nscap) <= 512, (ndcap, nscap)

    DSTR = int(sum(ndcap))
    xoff = np.concatenate([[0], np.cumsum([2 * s for s in nscap])]).astype(int)
    soff = np.concatenate(
        [[0], np.cumsum([(nscap[j] // 128) * ndcap[j] for j in range(8)])]
    ).astype(int)
    xsoff = np.concatenate(
        [[0], np.cumsum([(nscap[j] // 128) * D for j in range(8)])]
    ).astype(int)

    # plain nodes: all device nodes that are not a dst of their cluster
    is_dst = np.zeros(N, dtype=bool)
    for c in range(C):
        is_dst[clusters[c][0]] = True
    node_dev = cdev[cl]
    plain_lists = [
        np.where((node_dev == d) & ~is_dst)[0] for d in range(M)
    ]
    max_plain = max(p.size for p in plain_lists)
    MAIN_ROWS = DSTR + _ceil_to(max(max_plain, 1), 512)

    # split the dst region at a cluster boundary so the first dst
    # matmuls only wait for half the region's load
    half = int(sum(ndcap[:4]))
    chunks = [(0, half), (half, DSTR - half)] if 0 < half < DSTR else [(0, DSTR)]
    r0 = DSTR
    while r0 < MAIN_ROWS:
        # small chunks at the end so the last store doesn't trail far
        rem = MAIN_ROWS - r0
        L = min(_CHUNK if rem > 3072 else (1024 if rem > 1024 else 512), rem)
        chunks.append((r0, L))
        r0 += L

    w_pack = np.ascontiguousarray(
        weight.reshape(2, 128, D).transpose(1, 0, 2).astype(bf)
    )
    biasf = np.ascontiguousarray(bias.reshape(2, 128).T.astype(np.float32))

    in_maps = []
    main_ids = []
    for d in range(M):
        rows = np.zeros((MAIN_ROWS, D), dtype=np.float32)
        mid = np.full(MAIN_ROWS, -1, dtype=np.int64)
        xts = np.zeros((128, int(xoff[8])), dtype=bf)
        st = np.zeros((128, int(soff[8])), dtype=bf)
        pos = 0
        for j in range(8):
            c = dev_clusters[d][j]
            dst_u, src_u, S = clusters[c]
            nd, ns = dst_u.size, src_u.size
            rows[pos : pos + nd] = X[dst_u] * (dinv[dst_u] ** 2)[:, None]
            mid[pos : pos + nd] = dst_u
            pos += ndcap[j]
            if nscap[j]:
                xs = np.zeros((nscap[j], D), dtype=np.float32)
                xs[:ns] = X[src_u]
                xts[:, xoff[j] : xoff[j + 1]] = (
                    xs.reshape(nscap[j], 2, 128)
                    .transpose(2, 1, 0)
                    .reshape(128, 2 * nscap[j])
                )
                Sp = np.zeros((nscap[j], ndcap[j]), dtype=np.float32)
                Sp[:ns, :nd] = S.T
                st[:, soff[j] : soff[j + 1]] = (
                    Sp.reshape(nscap[j] // 128, 128, ndcap[j])
                    .transpose(1, 0, 2)
                    .reshape(128, (nscap[j] // 128) * ndcap[j])
                )
        plain = plain_lists[d]
        rows[DSTR : DSTR + plain.size] = X[plain]
        mid[DSTR : DSTR + plain.size] = plain
        main_ids.append(mid)
        in_maps.append({
            "xtm": _pack_rows(np.ascontiguousarray(rows.astype(bf)), chunks),
            "xts": np.ascontiguousarray(xts.astype(bf)),
            "st": np.ascontiguousarray(st.astype(bf)),
            "w": w_pack,
            "biasf": biasf,
        })

    nc = _build_program(
        ndcap, nscap, MAIN_ROWS, chunks, xoff, soff, xsoff
    )
    results = _run_program(nc, in_maps)

    epc = np.bincount(cl[ed], minlength=C)
    active = epc[cl] > 0

    out = X.copy()
    for d in range(M):
        rows = _unpack_rows(np.asarray(results[d]["outt"]), chunks, MAIN_ROWS)
        mid = main_ids[d]
        sel = mid >= 0
        ids = mid[sel]
        act = active[ids]
        out[ids[act]] = rows[sel][act]
    return out
